# revision 38
# baseline (speedup 1.0000x reference)
"""MultiHeadAttention TRN2 Bass kernel (B=4 S=2048 E=1024 H=16).

Sharding: core c -> (batch b = c//2, head-half hh = c%2); Megatron-style
out-proj partials summed on host (+bo on host).

All-fp16 matmuls, fp32 PSUM. Global stream of 256 "gens" in 16 windows
(mc-major: w = 4*mc + p). Each gen: 2 score MMs in alternating PE row
groups (concurrent via array tiling, ~109ns/MM pair), one ACT exp over
[128, 2, 512] PSUM (~1.11us/gen sets the pace). PV for window w runs one
window later (lag-16) through an SBUF pt ring so PV never waits on ACT;
the last window compresses PV 2-per-gen so the tail is short. Spread
work (V/K/Q/out-proj) is split into ~0.9us half-items paced into gens by
a static plan; wk/wq DMAs are chunked and the first chunks + x tiles are
queued ahead of everything else. PSUM: sc [128,2,512] x2 bufs (4 banks)
+ pvA + pvB + spread x2 = 8. Denominators ride the 65th ones-column of
V'; divide via scratch-DRAM partition-broadcast + DVE reciprocal.
"""

import numpy as np

import concourse.bass as bass
import concourse.mybir as mybir
import concourse.tile as tile
from concourse import bacc

F32 = mybir.dt.float32
F16 = mybir.dt.float16
AF = mybir.ActivationFunctionType

B, S, E, H, D = 4, 2048, 1024, 16, 64
HS = 512
PAIRS = 4
MC = 512
NMC = S // MC       # 4
NKT = E // 128      # 8
NMT = S // 128      # 16
NWIN = NMC * PAIRS  # 16 windows, mc-major
NGEN = NWIN * NMT   # 256


def build_nc():
    nc = bacc.Bacc()

    xq_d = nc.dram_tensor("xq_t", [E, S], F16, kind="ExternalInput")
    xk_d = nc.dram_tensor("xk_t", [E, S], F16, kind="ExternalInput")
    xv_d = nc.dram_tensor("xv_t", [E, S], F16, kind="ExternalInput")
    wq_d = nc.dram_tensor("wq_t", [E, HS], F16, kind="ExternalInput")
    wk_d = nc.dram_tensor("wk_t", [E, HS], F16, kind="ExternalInput")
    wv_d = nc.dram_tensor("wv_t", [E, HS], F16, kind="ExternalInput")
    wo_d = nc.dram_tensor("wo_t", [HS, E], F16, kind="ExternalInput")
    bq_d = nc.dram_tensor("bq", [HS], F32, kind="ExternalInput")
    bk_d = nc.dram_tensor("bk", [HS], F32, kind="ExternalInput")
    bv_d = nc.dram_tensor("bv_row", [1, HS], F16, kind="ExternalInput")

    out_d = nc.dram_tensor("out_partial", [S, E], F16, kind="ExternalOutput")
    scratch_d = nc.dram_tensor("scratch", [NMC, PAIRS, 2, MC], F32)

    def bcast_ap(row_ap, n):
        return bass.AP(tensor=row_ap.tensor, offset=row_ap.offset,
                       ap=[[0, n]] + list(row_ap.ap[1:]))

    with tile.TileContext(nc) as tc:
        with (
            tc.tile_pool(name="const", bufs=1) as const,
            tc.tile_pool(name="qkv", bufs=1) as qkv,
            tc.tile_pool(name="aout", bufs=1) as aoutp,
            tc.tile_pool(name="w", bufs=1) as wpool,
            tc.tile_pool(name="x", bufs=2) as xpool,
            tc.tile_pool(name="pt", bufs=16) as ptp,
            tc.tile_pool(name="msc", bufs=2) as msc,
            tc.tile_pool(name="ost", bufs=2) as ostp,
            tc.tile_pool(name="sc", bufs=2, space=bass.MemorySpace.PSUM) as scp,
            tc.tile_pool(name="pv", bufs=1, space=bass.MemorySpace.PSUM) as pvp,
            tc.tile_pool(name="sp", bufs=2, space=bass.MemorySpace.PSUM) as spp,
        ):
            # weight tiles; DMAs chunked per 128-col group, emitted lazily
            wk_sb = wpool.tile([128, NKT, HS], F16, tag="wk")
            wq_sb = wpool.tile([128, NKT, HS], F16, tag="wq")
            wv_sb = wpool.tile([128, NKT, HS], F16, tag="wv")
            wo_sb = wpool.tile([128, PAIRS, E], F16, tag="wo")
            wloaded = set()

            def load_w(which, nt):
                if (which, nt) in wloaded:
                    return
                wloaded.add((which, nt))
                sb, dr = {"k": (wk_sb, wk_d), "q": (wq_sb, wq_d),
                          "v": (wv_sb, wv_d)}[which]
                nc.sync.dma_start(
                    sb[:, :, nt * 128:(nt + 1) * 128],
                    dr.rearrange("(kc p) n -> p kc n", p=128)[
                        :, :, nt * 128:(nt + 1) * 128],
                )

            xcache = {}

            def stage_x(key, dram, mc):
                if key in xcache:
                    return xcache[key]
                x_t = xpool.tile([128, NKT, MC], F16, tag=key[0], bufs=2,
                                 name=f"x_{key[0]}{mc}_{key[-1]}")
                nc.sync.dma_start(
                    x_t[:],
                    dram.rearrange("(kc p) m -> p kc m", p=128)[
                        :, :, mc * MC:(mc + 1) * MC],
                )
                xcache[key] = x_t
                return x_t

            load_w("k", 0)
            load_w("k", 1)
            stage_x(("k", 0, "up"), xk_d, 0)
            stage_x(("k", 1, "up"), xk_d, 1)
            load_w("q", 0)
            load_w("q", 1)
            stage_x(("q", 0, "up"), xq_d, 0)

            ones_f16 = const.tile([1, 128], F16)
            nc.vector.memset(ones_f16[:], 1.0)
            bq_sb = const.tile([128, PAIRS], F32)
            nc.sync.dma_start(bq_sb[:], bq_d.rearrange("(t p) -> p t", p=128))
            bk_sb = const.tile([128, PAIRS], F32)
            nc.sync.dma_start(bk_sb[:], bk_d.rearrange("(t p) -> p t", p=128))
            bv_sb = const.tile([1, HS], F16)
            nc.sync.dma_start(bv_sb[:], bv_d[:])

            qt_all = qkv.tile([128, PAIRS, S], F16, tag="qt")
            kt_all = qkv.tile([128, PAIRS, S], F16, tag="kt")
            v_all = qkv.tile([128, NMT, 8, 65], F16, tag="v")
            nc.vector.memset(v_all[:, :, :, 64], 1.0)

            aout = [aoutp.tile([128, S], F16, name=f"aout{p}", tag=f"ao{p}")
                    for p in range(PAIRS)]

            # ---- spread items, each split into two halves -------------
            open_ps = {}

            def kqproj_half(which, mck, nt, half, xkey):
                w_sb, bias = ((wk_sb, bk_sb) if which == "k"
                              else (wq_sb, bq_sb))
                kt = kt_all if which == "k" else qt_all
                x_t = stage_x(xkey, xk_d if which == "k" else xq_d, mck)
                pk = (which, mck, nt)
                if half == 0:
                    open_ps[pk] = spp.tile([128, MC], F32, tag="sp",
                                           name=f"{which}p{mck}{nt}")
                ps = open_ps[pk]
                for kc in range(4 * half, 4 * half + 4):
                    nc.tensor.matmul(
                        ps[:],
                        w_sb[:, kc, nt * 128:(nt + 1) * 128],
                        x_t[:, kc, :],
                        start=(kc == 0), stop=(kc == NKT - 1),
                    )
                if half == 1:
                    del open_ps[pk]
                    nc.vector.tensor_scalar_add(
                        kt[:, nt, mck * MC:(mck + 1) * MC],
                        ps[:], bias[:, nt:nt + 1],
                    )

            def vproj_half(mt, half):
                mcv, mt_l = mt // (MC // 128), mt % (MC // 128)
                x_t = stage_x(("v", mcv), xv_d, mcv)
                if mcv + 1 < NMC and half == 0 and mt_l == 0:
                    stage_x(("v", mcv + 1), xv_d, mcv + 1)  # prefetch
                pk = ("v", mt)
                if half == 0:
                    open_ps[pk] = spp.tile([128, HS], F32, tag="sp",
                                           name=f"vp{mt}")
                ps = open_ps[pk]
                for kc in range(4 * half, 4 * half + 4):
                    nc.tensor.matmul(
                        ps[:],
                        x_t[:, kc, mt_l * 128:(mt_l + 1) * 128],
                        wv_sb[:, kc, :],
                        start=(kc == 0), stop=False,
                    )
                if half == 1:
                    del open_ps[pk]
                    nc.tensor.matmul(
                        ps[:], ones_f16[:], bv_sb[:], start=False, stop=True,
                    )
                    nc.vector.tensor_copy(
                        v_all[:, mt, :, 0:64],
                        ps[:].rearrange("p (h c) -> p h c", c=64),
                    )

            def outproj_half(mc, mt_l, nchunk, half):
                msl = slice(mc * MC + mt_l * 128, mc * MC + (mt_l + 1) * 128)
                nsl = slice(nchunk * 512, (nchunk + 1) * 512)
                pk = ("o", mc, mt_l, nchunk)
                if half == 0:
                    open_ps[pk] = spp.tile([128, 512], F32, tag="sp",
                                           name=f"op{mc}{mt_l}{nchunk}")
                ps = open_ps[pk]
                for dk in (2 * half, 2 * half + 1):
                    nc.tensor.matmul(
                        ps[:],
                        aout[dk][:, msl],
                        wo_sb[:, dk, nsl],
                        start=(dk == 0), stop=(dk == PAIRS - 1),
                    )
                if half == 1:
                    del open_ps[pk]
                    ost = ostp.tile([128, 512], F16, tag="ost")
                    nc.vector.tensor_copy(ost[:], ps[:])
                    nc.sync.dma_start(out_d[msl, nsl], ost[:])

            # ---- static spread plan ----------------------------------
            plan = [[] for _ in range(NGEN + 1)]

            def addplan(g, fn):
                plan[min(NGEN, g)].append(fn)

            # V-proj: tiles 0-11 over window 0, tiles 12-15 in window 1
            g = 0
            for mt in range(12):
                for half in (0, 1):
                    addplan(g * 2 // 3, lambda mt=mt, h=half: vproj_half(mt, h))
                    g += 1
            for mt in range(12, NMT):
                for half in (0, 1):
                    addplan(16 + (mt - 12) * 2 + half,
                            lambda mt=mt, h=half: vproj_half(mt, h))
            # K-proj nt=2 in window 1 gens 8-15, nt=3 in window 2
            addplan(4, lambda: load_w("k", 2))
            addplan(20, lambda: load_w("k", 3))
            for nt, gg in ((2, 24), (3, 33)):
                for j, mck in enumerate(range(NMC)):
                    for half in (0, 1):
                        addplan(gg + 2 * j + half,
                                lambda mck=mck, nt=nt, h=half: kqproj_half(
                                    "k", mck, nt, h, ("k", mck, nt)))
            # Q-proj: (0,0),(0,1) upfront; rest as halves before deadline
            addplan(8, lambda: load_w("q", 2))
            addplan(24, lambda: load_w("q", 3))
            for mc in range(NMC):
                for p in range(PAIRS):
                    if mc == 0 and p < 2:
                        continue
                    dl = 16 * (mc * PAIRS + p)
                    gg = {32: 29, 48: 41}.get(dl, dl - 20)
                    for half in (0, 1):
                        addplan(gg + half,
                                lambda mc=mc, p=p, h=half: kqproj_half(
                                    "q", mc, p, h, ("q", mc, p)))
            # out-proj: aout(mc) done at end of window 4*mc+5
            def load_wo():
                nc.sync.dma_start(
                    wo_sb[:], wo_d.rearrange("(dk p) n -> p dk n", p=128))
            addplan(60, load_wo)
            for mc in range(NMC):
                base = 16 * (4 * mc + 5)
                for j, (mt_l, nk) in enumerate(
                        [(m, n) for m in range(MC // 128) for n in range(2)]):
                    for half in (0, 1):
                        addplan(base + 2 * j + half + (0 if mc < 3 else 1),
                                lambda mc=mc, m=mt_l, n=nk, h=half:
                                outproj_half(mc, m, n, h))

            # ---- upfront phase ---------------------------------------
            for mck in range(NMC):
                for nt in (0, 1):
                    for half in (0, 1):
                        kqproj_half("k", mck, nt, half, ("k", mck, "up"))
            for p in (0, 1):
                for half in (0, 1):
                    kqproj_half("q", 0, p, half, ("q", 0, "up"))
            # wv + first xv for window-0 V-proj spread
            for nt in range(PAIRS):
                load_w("v", nt)
            stage_x(("v", 0), xv_d, 0)

            # ---- gen loop --------------------------------------------
            pts = {}
            pvacc = {}

            def pv_pair(w, t):
                p = w % PAIRS
                if t == 0:
                    pvacc[w] = (
                        pvp.tile([128, MC], F32, tag="pvA", name=f"pvA{w}"),
                        pvp.tile([128, MC], F32, tag="pvB", name=f"pvB{w}"),
                    )
                pvA, pvB = pvacc[w]
                pt_t = pts.pop((w, t))
                nc.tensor.matmul(
                    pvA[0:65, :], v_all[:, t, 2 * p, :], pt_t[:, 0, :],
                    start=(t == 0), stop=(t == NMT - 1),
                )
                nc.tensor.matmul(
                    pvB[0:65, :], v_all[:, t, 2 * p + 1, :], pt_t[:, 1, :],
                    start=(t == 0), stop=(t == NMT - 1),
                )

            def divides(w):
                mc, p = w // PAIRS, w % PAIRS
                m1 = slice(mc * MC, (mc + 1) * MC)
                pvA, pvB = pvacc.pop(w)
                for h, pvt in ((0, pvA), (1, pvB)):
                    pvs = msc.tile([128, MC], F32, name=f"pvs{w}{h}",
                                   tag="pvs")
                    nc.vector.tensor_copy(pvs[0:65, :], pvt[0:65, :])
                    srow_dram = scratch_d[mc:mc + 1, p, h, :]
                    nc.sync.dma_start(srow_dram, pvs[64:65, :])
                    bc = msc.tile([64, MC], F32, tag="bc")
                    nc.sync.dma_start(bc[:], bcast_ap(srow_dram, 64))
                    inv = msc.tile([64, MC], F32, tag="inv")
                    nc.vector.reciprocal_approx_fast(inv[:], bc[:])
                    if h == 0:
                        nc.vector.tensor_mul(
                            aout[p][0:64, m1], pvs[0:64, :], inv[:])
                    else:
                        tmpb = msc.tile([64, MC], F16, tag="tmpb")
                        nc.vector.tensor_mul(tmpb[:], pvs[0:64, :], inv[:])
                        nc.sync.dma_start(aout[p][64:128, m1], tmpb[:])

            for w in range(NWIN):
                mc, p = w // PAIRS, w % PAIRS
                m1 = slice(mc * MC, (mc + 1) * MC)
                last = (w == NWIN - 1)
                for t in range(NMT):
                    g = w * NMT + t
                    m2 = slice(t * 128, (t + 1) * 128)
                    sc = scp.tile([128, 2, MC], F32, tag="sc", name=f"sc{g}")
                    nc.tensor.matmul(
                        sc[:, 0, :],
                        kt_all[0:64, p, m2], qt_all[0:64, p, m1],
                        start=True, stop=True, tile_position=(0, 0),
                    )
                    nc.tensor.matmul(
                        sc[:, 1, :],
                        kt_all[64:128, p, m2], qt_all[64:128, p, m1],
                        start=True, stop=True, tile_position=(64, 0),
                    )
                    pt_t = ptp.tile([128, 2, MC], F16, tag="pt", name=f"pt{g}")
                    nc.scalar.activation(pt_t[:], sc[:], AF.Exp, scale=0.125)
                    pts[(w, t)] = pt_t
                    if not last:
                        if w >= 1:
                            pv_pair(w - 1, t)
                            if t == NMT - 1:
                                divides(w - 1)
                    else:
                        if t < 8:
                            pv_pair(w - 1, 2 * t)
                            pv_pair(w - 1, 2 * t + 1)
                            if t == 7:
                                divides(w - 1)
                        else:
                            pv_pair(w, 2 * (t - 8))
                            pv_pair(w, 2 * (t - 8) + 1)
                    for work in plan[g]:
                        work()
            divides(NWIN - 1)
            for work in plan[NGEN]:
                work()

    return nc


def kernel(**inputs):
    query = np.asarray(inputs["query"], np.float32)
    key = np.asarray(inputs["key"], np.float32)
    value = np.asarray(inputs["value"], np.float32)
    Wq = np.asarray(inputs["Wq"], np.float32)
    bq = np.asarray(inputs["bq"], np.float32)
    Wk = np.asarray(inputs["Wk"], np.float32)
    bk = np.asarray(inputs["bk"], np.float32)
    Wv = np.asarray(inputs["Wv"], np.float32)
    bv = np.asarray(inputs["bv"], np.float32)
    Wo = np.asarray(inputs["Wo"], np.float32)
    bo = np.asarray(inputs["bo"], np.float32)

    nc = build_nc()

    in_maps = []
    for c in range(8):
        b, hh = c // 2, c % 2
        hs = slice(hh * HS, (hh + 1) * HS)

        def prep(a):
            return np.ascontiguousarray(a).astype(np.float16)

        in_maps.append({
            "xq_t": prep(query[b].T),
            "xk_t": prep(key[b].T),
            "xv_t": prep(value[b].T),
            "wq_t": prep(Wq[hs, :].T),
            "wk_t": prep(Wk[hs, :].T),
            "wv_t": prep(Wv[hs, :].T),
            "wo_t": prep(Wo[:, hs].T),
            "bq": np.ascontiguousarray(bq[hs]),
            "bk": np.ascontiguousarray(bk[hs]),
            "bv_row": prep(bv[hs].reshape(1, HS)),
        })

    from concourse.bass_utils import run_bass_kernel_spmd
    nc.finalize()
    r = run_bass_kernel_spmd(nc, in_maps, core_ids=list(range(8)))
    globals()["LAST_RUN"] = r
    outs = [r.results[c]["out_partial"].astype(np.float32) for c in range(8)]
    return np.stack([outs[2 * b] + outs[2 * b + 1] for b in range(B)]) + bo
